# revision 18
# baseline (speedup 1.0000x reference)
"""Multi-head causal attention (B=2, T=2048, C=1024, H=16) on 8 Trainium2
NeuronCores, tensor-parallel over heads (2 heads per core).

v2 — engine-balanced redesign of the v1 flash kernel:
  - scores: the two heads' K^T@Q matmuls are issued adjacently with base
    partitions 0/64 so they land on different PE row groups and run
    CONCURRENTLY (row tiling), into the two halves of one [128,1024] PSUM
    pair tile.
  - causal mask: added on the PE as a tiny N=128 matmul (identity @ mtri)
    accumulated into the diagonal 128-block of the scores group — replaces
    the expensive [128,512] DVE adds.
  - exp: one ScalarE ACTIVATE per k-tile covering BOTH heads via a 3D AP
    over the [128,1024] pair (halves ScalarE instruction count).
  - diagonal k-tiles only compute the valid q-range (scores, exp, AV all
    use free dim 512-128*v).
  - AV keeps the ones-column trick (stationary [128,65] Vaug) for free
    softmax denominators; both heads' O accumulate across the whole chunk
    in two single-buffered PSUM banks.
  - normalize: rowsums -> reciprocal on [2,512] -> one fp32r broadcast
    matmul -> ScalarE copy -> two DVE muls into a combined ots[128,512]
    (h0 on partitions 0-63, h1 on 64-127).
  - out-projection: single full-contract matmul per m-tile (contract over
    both heads at once), DVE copy to bf16, DMA out.
  - x input host-packed as [p, chunk, ktile, t] so each chunk's load is one
    DMA with 8KB contiguous lines; y output in bf16 (halves write traffic).
"""

import os
import sys

for _p in ("/opt/trn_rl_repo", "/root/.axon_site/_ro/trn_rl_repo"):
    if os.path.isdir(_p) and _p not in sys.path:
        sys.path.insert(0, _p)

import ml_dtypes
import numpy as np

import concourse.bacc as bacc
import concourse.bass as bass
import concourse.mybir as mybir
import concourse.tile as tile
from concourse.bass_utils import run_bass_kernel_spmd
from concourse.masks import make_identity

B, T, C, H, D = 2, 2048, 1024, 16, 64
NCORES = 8
BT = B * T                      # 4096 flattened tokens
TC = 512                        # token chunk (matmul free dim)
NTC = BT // TC                  # 8 token chunks
FP = mybir.dt.float32
FPR = mybir.dt.float32r
BF = mybir.dt.bfloat16
ACT = mybir.ActivationFunctionType
NEG = -1.0e9
AV_DELAY = 3                    # k-tiles the AV matmul trails the scores

LAST_RESULTS = None             # stashed BassKernelResults for test harness


def build_nc():
    nc = bacc.Bacc(None, target_bir_lowering=False, debug=False)

    xh = nc.declare_dram_parameter("xh", [128, NTC * 4096], BF, isOutput=False)
    wc = nc.declare_dram_parameter("wc", [C, 384], BF, isOutput=False)
    wout = nc.declare_dram_parameter("wout", [128, C], BF, isOutput=False)
    bqkv = nc.declare_dram_parameter("bqkv", [128, 3], FP, isOutput=False)
    mtri = nc.declare_dram_parameter("mtri", [128, 128], BF, isOutput=False)
    ones = nc.declare_dram_parameter("ones", [128, 64], BF, isOutput=False)
    onesr = nc.declare_dram_parameter("onesr", [1, 64], FP, isOutput=False)
    yh = nc.declare_dram_parameter("yh", [128, NTC * 4096], BF, isOutput=True)

    with tile.TileContext(nc) as tc:
        with (
            tc.tile_pool(name="const", bufs=1) as cpool,
            tc.tile_pool(name="big", bufs=1) as bigpool,
            tc.tile_pool(name="sb", bufs=2) as sbpool,
            tc.tile_pool(name="ps", bufs=2, space="PSUM") as pspool,
        ):
            # ---- constants ----
            wc_sb = cpool.tile([128, 8 * 384], BF)      # [cin, k*384 + g*128 + col]
            nc.sync.dma_start(
                out=wc_sb[:].rearrange("b (a c) -> b a c", a=8),
                in_=wc.rearrange("(a b) c -> b a c", a=8),
            )
            wout_sb = cpool.tile([128, C], BF)          # rows: h0 d0-63 | h1 d0-63
            nc.sync.dma_start(out=wout_sb[:], in_=wout[:, :])
            bq_sb = cpool.tile([128, 3], FP)
            nc.sync.dma_start(out=bq_sb[:], in_=bqkv[:, :])
            mtri_sb = cpool.tile([128, 128], BF)
            nc.sync.dma_start(out=mtri_sb[:], in_=mtri[:, :])
            onesr_sb = cpool.tile([1, 64], FPR)
            nc.sync.dma_start(out=onesr_sb[:], in_=onesr.bitcast(FPR)[:, :])
            ident = cpool.tile([128, 128], BF)
            make_identity(nc, ident)

            # ---- persistent intermediates ----
            QT = bigpool.tile([128, BT], BF)
            KT = bigpool.tile([128, BT], BF)
            VT = bigpool.tile([128, BT], BF)
            # V in [token, dim] layout, 130 cols per 128-token block:
            # [V_h0 (64) | ones | V_h1 (64) | ones]
            vaug = bigpool.tile([128, 32 * 130], BF)
            ones_sb = cpool.tile([128, 64], BF)
            nc.sync.dma_start(out=ones_sb[:], in_=ones[:, :])
            nc.vector.tensor_copy(
                vaug[:].rearrange("p (j a c) -> p j a c", a=2, c=65)[
                    :, :, :, 64:65],
                ones_sb[:].rearrange("p (j a c) -> p j a c", a=2, c=1)[:, 0:32],
            )

            qkvT = (QT, KT, VT)
            state = {"pending": None}   # (otp, rc2, t0) awaiting norm+proj

            def emit_qkv(tcx):
                t0 = tcx * TC
                xtile = sbpool.tile([128, 4096], BF, tag="xt", bufs=3,
                                    name="xtile")
                nc.sync.dma_start(
                    out=xtile[:], in_=xh[:, tcx * 4096:(tcx + 1) * 4096]
                )
                for g in range(3):
                    qp = pspool.tile([128, TC], FP, tag="t", bufs=2, name="qp")
                    for k in range(8):
                        nc.tensor.matmul(
                            qp[:],
                            wc_sb[:, k * 384 + g * 128:k * 384 + (g + 1) * 128],
                            xtile[:, k * TC:(k + 1) * TC],
                            start=(k == 0),
                            stop=(k == 7),
                        )
                    nc.vector.tensor_scalar_add(
                        qkvT[g][:, t0:t0 + TC], qp[:], bq_sb[:, g:g + 1],
                    )
                # transpose this chunk's V into vaug
                for j in range(4):
                    jj = tcx * 4 + j
                    tpf = pspool.tile([128, TC], FP, tag="t", bufs=2,
                                      name="tp")
                    tp = tpf.bitcast(BF)[:, 0:128]
                    nc.tensor.transpose(
                        tp, VT[:, jj * 128:(jj + 1) * 128], ident[:]
                    )
                    nc.vector.tensor_copy(
                        vaug[:].rearrange("p (j a c) -> p j a c", a=2, c=65)[
                            :, jj, :, 0:64],
                        tp.rearrange("p (a c) -> p a c", c=64),
                    )

            def emit_norm(otp_prev, rc2_prev):
                """normalize the previous chunk: fp32r broadcast matmuls of
                the rowsums, reciprocal, scale O -> ots (bf16)."""
                bcps = []
                for h in range(2):
                    bcp = pspool.tile([64, TC], FP, tag="t", bufs=2,
                                      name="bcp")
                    nc.tensor.matmul(
                        bcp[:], onesr_sb[:],
                        rc2_prev[0:1, h * TC:(h + 1) * TC],
                        start=True, stop=True, skip_group_check=True,
                    )
                    bcps.append(bcp)
                bcs = sbpool.tile([64, 2 * TC], FP, tag="bc", bufs=2,
                                  name="bcs")
                for h in range(2):
                    nc.vector.reciprocal_approx_fast(
                        out=bcs[:, h * TC:(h + 1) * TC], in_=bcps[h][:]
                    )
                ots = sbpool.tile([128, TC], BF, tag="ot", bufs=2, name="ots")
                nc.vector.tensor_mul(ots[0:64, :], otp_prev[0:64, 0:TC],
                                     bcs[0:64, 0:TC])
                nc.vector.tensor_mul(ots[64:128, :], otp_prev[0:64, TC:2 * TC],
                                     bcs[0:64, TC:2 * TC])
                return ots

            def emit_outproj(ots_prev, t0_prev):
                for m in range(8):
                    yp = pspool.tile([128, TC], FP, tag="t", bufs=2,
                                     name="yp")
                    nc.tensor.matmul(
                        yp[:], wout_sb[:, m * 128:(m + 1) * 128],
                        ots_prev[:], start=True, stop=True,
                    )
                    ysb = sbpool.tile([128, TC], BF, tag="ys", bufs=4,
                                      name="ysb")
                    if m % 2 == 0:
                        nc.scalar.copy(ysb[:], yp[:])
                    else:
                        nc.vector.tensor_copy(ysb[:], yp[:])
                    nc.sync.dma_start(
                        out=yh[:, (t0_prev // TC) * 4096
                               + m * TC:(t0_prev // TC) * 4096
                               + (m + 1) * TC],
                        in_=ysb[:],
                    )

            def emit_attn(tcx):
                b, qc = divmod(tcx, 4)
                t0 = tcx * TC
                n_kt = 4 * (qc + 1)
                otp = pspool.tile([65, 2 * TC], FP, tag="o", bufs=1,
                                  name="otp")
                pts = {}
                ots_prev = [None]

                def emit_av(j, kg0):
                    pt, qs = pts.pop(j)
                    for h in range(2):
                        nc.tensor.matmul(
                            otp[:, h * TC + qs:(h + 1) * TC],
                            vaug[:, kg0 * 130 + h * 65:kg0 * 130 + h * 65 + 65],
                            pt[:, h * TC + qs:(h + 1) * TC],
                            start=(j == 0), stop=(j == n_kt - 1),
                            skip_group_check=True,
                        )

                def inject(kt):
                    # previous chunk's normalize (kt 1) and out-projection
                    # (kt 3), delayed so the PE queue has fill work while
                    # the cross-engine chains complete
                    if kt == 1 and state["pending"] is not None:
                        otp_p, rc2_p, t0_p = state["pending"]
                        ots_prev[0] = (emit_norm(otp_p, rc2_p), t0_p)
                        state["pending"] = None
                    if kt == 3 and ots_prev[0] is not None:
                        emit_outproj(*ots_prev[0])
                        ots_prev[0] = None

                for kt in range(n_kt):
                    kg = b * 16 + kt
                    diag = kt >= 4 * qc
                    v = kt - 4 * qc if diag else 0
                    qs = v * 128
                    sp = pspool.tile([128, 2 * TC], FP, tag="s", bufs=2,
                                     name="sp")
                    for h in range(2):
                        nc.tensor.matmul(
                            sp[:, h * TC + qs:(h + 1) * TC],
                            KT[h * 64:(h + 1) * 64, kg * 128:(kg + 1) * 128],
                            QT[h * 64:(h + 1) * 64, t0 + qs:t0 + TC],
                            start=True, stop=not diag,
                            skip_group_check=True,
                        )
                    if diag:
                        for h in range(2):
                            nc.tensor.matmul(
                                sp[:, h * TC + qs:h * TC + qs + 128],
                                ident[:],
                                mtri_sb[:],
                                start=False, stop=True,
                                skip_group_check=True,
                            )
                    pt = sbpool.tile([128, 2 * TC], BF, tag="pt", bufs=5,
                                     name="pt")
                    nc.scalar.activation(
                        pt[:].rearrange("p (j q) -> p j q", j=2)[:, :, qs:TC],
                        sp[:].rearrange("p (j q) -> p j q", j=2)[:, :, qs:TC],
                        ACT.Exp, scale=0.125,
                    )
                    pts[kt] = (pt, qs)
                    inject(kt)
                    if kt >= AV_DELAY:
                        emit_av(kt - AV_DELAY, b * 16 + kt - AV_DELAY)
                for j in range(max(n_kt - AV_DELAY, 0), n_kt):
                    emit_av(j, b * 16 + j)

                # rowsum extraction (ScalarE, fp32r-rounded); the rest of
                # the normalize is deferred into the next attention
                rc2 = sbpool.tile([1, 2 * TC], FPR, tag="rc", bufs=2,
                                  name="rc2")
                with nc.allow_low_precision(reason="softmax sums f32r"):
                    nc.scalar.copy(rc2[:], otp[64:65, :])
                state["pending"] = (otp, rc2, t0)

            for b in range(2):
                base = b * 4
                emit_qkv(base + 0)
                emit_qkv(base + 1)
                emit_attn(base + 1)
                emit_qkv(base + 2)
                emit_attn(base + 2)
                emit_qkv(base + 3)
                emit_attn(base + 3)
                emit_attn(base + 0)

            otp_p, rc2_p, t0_p = state["pending"]
            emit_outproj(emit_norm(otp_p, rc2_p), t0_p)
    nc.compile()
    return nc


def make_in_maps(x, w_qkv, b_qkv, w_out):
    x = np.ascontiguousarray(np.asarray(x, np.float32).reshape(BT, C))
    xT = np.ascontiguousarray(x.T)                    # [C, BT]
    # [a(8), p(128), tcx(8), t(512)] -> [p, tcx, a, t]
    xhp = np.ascontiguousarray(
        xT.reshape(8, 128, NTC, TC).transpose(1, 2, 0, 3).reshape(128, -1)
    ).astype(ml_dtypes.bfloat16)
    w_qkv = np.asarray(w_qkv, np.float32)
    b_qkv = np.asarray(b_qkv, np.float32)
    w_out = np.asarray(w_out, np.float32)

    kk = np.arange(128)[:, None]
    qq = np.arange(128)[None, :]
    mtri = np.where(kk <= qq, 0.0, NEG).astype(ml_dtypes.bfloat16)

    in_maps = []
    for c in range(NCORES):
        sl = slice(c * 128, (c + 1) * 128)
        wcs = np.concatenate(
            [w_qkv[:, sl], w_qkv[:, 1024:][:, sl], w_qkv[:, 2048:][:, sl]],
            axis=1,
        )
        bq = np.stack(
            [b_qkv[sl], b_qkv[1024:][sl], b_qkv[2048:][sl]], axis=1
        )
        in_maps.append({
            "xh": xhp,
            "wc": np.ascontiguousarray(wcs).astype(ml_dtypes.bfloat16),
            "wout": np.ascontiguousarray(w_out[sl, :]).astype(
                ml_dtypes.bfloat16),
            "bqkv": np.ascontiguousarray(bq),
            "mtri": mtri,
            "ones": np.ones((128, 64), ml_dtypes.bfloat16),
            "onesr": np.ones((1, 64), np.float32),
        })
    return in_maps


_NC_CACHE = None


def kernel(x, w_qkv, b_qkv, w_out, b_out):
    global _NC_CACHE, LAST_RESULTS
    if _NC_CACHE is None:
        _NC_CACHE = build_nc()
    nc = _NC_CACHE

    in_maps = make_in_maps(x, w_qkv, b_qkv, w_out)

    res = run_bass_kernel_spmd(
        nc, in_maps, list(range(NCORES)),
        trace=bool(os.environ.get("BASS_TRACE")),
    )
    LAST_RESULTS = res

    acc = np.zeros((C, BT), np.float32)
    for out_map in res.results:
        # yh [p, tcx(8), m(8), t(512)] -> [m, p, tcx, t] -> [C, BT]
        yc = np.asarray(out_map["yh"]).reshape(128, NTC, 8, TC)
        acc += yc.transpose(2, 0, 1, 3).reshape(C, BT).astype(np.float32)
    y = acc.T + np.asarray(b_out, np.float32)[None, :]
    return y.reshape(B, T, C)


# revision 22
# speedup vs baseline: 1.0443x; 1.0443x over previous
"""Multi-head causal attention (B=2, T=2048, C=1024, H=16) on 8 Trainium2
NeuronCores, tensor-parallel over heads (2 heads per core).

v2 — engine-balanced redesign of the v1 flash kernel:
  - scores: the two heads' K^T@Q matmuls are issued adjacently with base
    partitions 0/64 so they land on different PE row groups and run
    CONCURRENTLY (row tiling), into the two halves of one [128,1024] PSUM
    pair tile.
  - causal mask: added on the PE as a tiny N=128 matmul (identity @ mtri)
    accumulated into the diagonal 128-block of the scores group — replaces
    the expensive [128,512] DVE adds.
  - exp: one ScalarE ACTIVATE per k-tile covering BOTH heads via a 3D AP
    over the [128,1024] pair (halves ScalarE instruction count).
  - diagonal k-tiles only compute the valid q-range (scores, exp, AV all
    use free dim 512-128*v).
  - AV keeps the ones-column trick (stationary [128,65] Vaug) for free
    softmax denominators; both heads' O accumulate across the whole chunk
    in two single-buffered PSUM banks.
  - normalize: rowsums -> reciprocal on [2,512] -> one fp32r broadcast
    matmul -> ScalarE copy -> two DVE muls into a combined ots[128,512]
    (h0 on partitions 0-63, h1 on 64-127).
  - out-projection: single full-contract matmul per m-tile (contract over
    both heads at once), DVE copy to bf16, DMA out.
  - x input host-packed as [p, chunk, ktile, t] so each chunk's load is one
    DMA with 8KB contiguous lines; y output in bf16 (halves write traffic).
"""

import os
import sys

for _p in ("/opt/trn_rl_repo", "/root/.axon_site/_ro/trn_rl_repo"):
    if os.path.isdir(_p) and _p not in sys.path:
        sys.path.insert(0, _p)

import ml_dtypes
import numpy as np

import concourse.bacc as bacc
import concourse.bass as bass
import concourse.mybir as mybir
import concourse.tile as tile
from concourse.bass_utils import run_bass_kernel_spmd
from concourse.masks import make_identity

B, T, C, H, D = 2, 2048, 1024, 16, 64
NCORES = 8
BT = B * T                      # 4096 flattened tokens
TC = 512                        # token chunk (matmul free dim)
NTC = BT // TC                  # 8 token chunks
FP = mybir.dt.float32
FPR = mybir.dt.float32r
BF = mybir.dt.bfloat16
ACT = mybir.ActivationFunctionType
NEG = -1.0e9
AV_DELAY = 3                    # k-tiles the AV matmul trails the scores

LAST_RESULTS = None             # stashed BassKernelResults for test harness


def build_nc():
    nc = bacc.Bacc(None, target_bir_lowering=False, debug=False)

    xh = nc.declare_dram_parameter("xh", [128, NTC * 4096], BF, isOutput=False)
    wc = nc.declare_dram_parameter("wc", [C, 384], BF, isOutput=False)
    wout = nc.declare_dram_parameter("wout", [128, C], BF, isOutput=False)
    bqkv = nc.declare_dram_parameter("bqkv", [128, 3], FP, isOutput=False)
    mtri = nc.declare_dram_parameter("mtri", [128, 128], BF, isOutput=False)
    ones = nc.declare_dram_parameter("ones", [128, 64], BF, isOutput=False)
    onesr = nc.declare_dram_parameter("onesr", [1, 64], FP, isOutput=False)
    yh = nc.declare_dram_parameter("yh", [128, NTC * 4096], BF, isOutput=True)

    with tile.TileContext(nc) as tc:
        with (
            tc.tile_pool(name="const", bufs=1) as cpool,
            tc.tile_pool(name="big", bufs=1) as bigpool,
            tc.tile_pool(name="sb", bufs=2) as sbpool,
            tc.tile_pool(name="ps", bufs=2, space="PSUM") as pspool,
        ):
            # ---- constants (wc group-0 + first x chunk lead the DMA ring
            # so the first matmuls can start ASAP) ----
            wc_sb = cpool.tile([128, 8 * 384], BF)      # [cin, k*384 + g*128 + col]
            wc_r = wc.rearrange("(a b) (g c) -> b a g c", a=8, g=3)
            wc_v = wc_sb[:].rearrange("b (a g c) -> b a g c", a=8, g=3)
            nc.sync.dma_start(out=wc_v[:, :, 0], in_=wc_r[:, :, 0])
            bq_sb = cpool.tile([128, 3], FP)
            nc.sync.dma_start(out=bq_sb[:], in_=bqkv[:, :])
            ident = cpool.tile([128, 128], BF)
            make_identity(nc, ident)

            # first chunk's x, split in halves so k-tile 0 lands early
            xtile0 = sbpool.tile([128, 4096], BF, tag="xt", bufs=3,
                                 name="xtile0")
            nc.sync.dma_start(out=xtile0[:, 0:2048], in_=xh[:, 0:2048])
            nc.sync.dma_start(out=wc_v[:, :, 1:3], in_=wc_r[:, :, 1:3])
            nc.sync.dma_start(out=xtile0[:, 2048:4096], in_=xh[:, 2048:4096])

            wout_sb = cpool.tile([128, C], BF)          # rows: h0 d | h1 d
            nc.sync.dma_start(out=wout_sb[:], in_=wout[:, :])
            mtri_sb = cpool.tile([128, 128], BF)
            nc.sync.dma_start(out=mtri_sb[:], in_=mtri[:, :])
            onesr_sb = cpool.tile([1, 64], FPR)
            nc.sync.dma_start(out=onesr_sb[:], in_=onesr.bitcast(FPR)[:, :])

            # ---- persistent intermediates ----
            QT = bigpool.tile([128, BT], BF)
            KT = bigpool.tile([128, BT], BF)
            VT = bigpool.tile([128, BT], BF)
            # V in [token, dim] layout, 130 cols per 128-token block:
            # [V_h0 (64) | ones | V_h1 (64) | ones]
            vaug = bigpool.tile([128, 32 * 130], BF)
            ones_sb = cpool.tile([128, 64], BF)
            nc.sync.dma_start(out=ones_sb[:], in_=ones[:, :])
            nc.vector.tensor_copy(
                vaug[:].rearrange("p (j a c) -> p j a c", a=2, c=65)[
                    :, :, :, 64:65],
                ones_sb[:].rearrange("p (j a c) -> p j a c", a=2, c=1)[:, 0:32],
            )

            qkvT = (QT, KT, VT)
            state = {"pending": None}   # (otp, rc2, t0) awaiting norm+proj

            def qkv_group(tcx, xtile, g):
                qp = pspool.tile([128, TC], FP, tag="t", bufs=2, name="qp")
                for k in range(8):
                    nc.tensor.matmul(
                        qp[:],
                        wc_sb[:, k * 384 + g * 128:k * 384 + (g + 1) * 128],
                        xtile[:, k * TC:(k + 1) * TC],
                        start=(k == 0),
                        stop=(k == 7),
                    )
                nc.vector.tensor_scalar_add(
                    qkvT[g][:, tcx * TC:(tcx + 1) * TC], qp[:],
                    bq_sb[:, g:g + 1],
                )

            def v_transpose(tcx, j):
                jj = tcx * 4 + j
                tpf = pspool.tile([128, TC], FP, tag="t", bufs=2, name="tp")
                tp = tpf.bitcast(BF)[:, 0:128]
                nc.tensor.transpose(
                    tp, VT[:, jj * 128:(jj + 1) * 128], ident[:]
                )
                nc.vector.tensor_copy(
                    vaug[:].rearrange("p (j a c) -> p j a c", a=2, c=65)[
                        :, jj, :, 0:64],
                    tp.rearrange("p (a c) -> p a c", c=64),
                )

            def qkv_units(tcx, xtile_pre=None):
                """fill-units (callables) computing chunk tcx's Q/K/V."""
                box = {"xt": xtile_pre}

                def dma():
                    if box["xt"] is None:
                        xt_ = sbpool.tile([128, 4096], BF, tag="xt", bufs=3,
                                          name="xtile")
                        nc.sync.dma_start(
                            out=xt_[:],
                            in_=xh[:, tcx * 4096:(tcx + 1) * 4096],
                        )
                        box["xt"] = xt_

                units = [dma]
                for g in range(3):
                    units.append(
                        lambda g=g: qkv_group(tcx, box["xt"], g))
                units.append(lambda: (v_transpose(tcx, 0),
                                      v_transpose(tcx, 1)))
                units.append(lambda: (v_transpose(tcx, 2),
                                      v_transpose(tcx, 3)))
                return units

            def emit_norm(otp_prev, rc2_prev):
                """normalize a finished chunk: fp32r broadcast matmuls of
                the rowsums, reciprocal, scale O -> ots (bf16)."""
                bcps = []
                for h in range(2):
                    bcp = pspool.tile([64, TC], FP, tag="t", bufs=2,
                                      name="bcp")
                    nc.tensor.matmul(
                        bcp[:], onesr_sb[:],
                        rc2_prev[0:1, h * TC:(h + 1) * TC],
                        start=True, stop=True, skip_group_check=True,
                    )
                    bcps.append(bcp)
                bcs = sbpool.tile([64, 2 * TC], FP, tag="bc", bufs=2,
                                  name="bcs")
                for h in range(2):
                    nc.vector.reciprocal_approx_fast(
                        out=bcs[:, h * TC:(h + 1) * TC], in_=bcps[h][:]
                    )
                ots = sbpool.tile([128, TC], BF, tag="ot", bufs=2, name="ots")
                nc.vector.tensor_mul(ots[0:64, :], otp_prev[0:64, 0:TC],
                                     bcs[0:64, 0:TC])
                nc.vector.tensor_mul(ots[64:128, :], otp_prev[0:64, TC:2 * TC],
                                     bcs[0:64, TC:2 * TC])
                return ots

            def outproj_m(ots_prev, t0_prev, m):
                yp = pspool.tile([128, TC], FP, tag="t", bufs=2, name="yp")
                nc.tensor.matmul(
                    yp[:], wout_sb[:, m * 128:(m + 1) * 128],
                    ots_prev[:], start=True, stop=True,
                )
                ysb = sbpool.tile([128, TC], BF, tag="ys", bufs=4, name="ysb")
                nc.vector.tensor_copy(ysb[:], yp[:])
                nc.sync.dma_start(
                    out=yh[:, (t0_prev // TC) * 4096
                           + m * TC:(t0_prev // TC) * 4096 + (m + 1) * TC],
                    in_=ysb[:],
                )

            def drain_units(extra):
                """fill units for the pending chunk's normalize+outproj,
                interleaved with `extra` (next chunk's qkv units)."""
                box = {}

                def norm():
                    otp_p, rc2_p, t0_p = state["pending"]
                    state["pending"] = None
                    box["ots"] = emit_norm(otp_p, rc2_p)
                    box["t0"] = t0_p

                projs = [lambda m=m: outproj_m(box["ots"], box["t0"], m)
                         for m in range(8)]
                ex = list(extra)
                noop = lambda: None
                # dma first (prefetch), norm next, a noop so the norm's
                # DVE chain completes before the first outproj matmul,
                # then alternate outproj with the qkv units
                units = [ex.pop(0) if ex else noop, norm, noop,
                         projs[0], projs[1]]
                rest = projs[2:]
                while ex or rest:
                    if ex:
                        units.append(ex.pop(0))
                    if rest:
                        units.append(rest.pop(0))
                return units

            def emit_attn(tcx, fill):
                b, qc = divmod(tcx, 4)
                t0 = tcx * TC
                n_kt = 4 * (qc + 1)
                otp = pspool.tile([65, 2 * TC], FP, tag="o", bufs=1,
                                  name="otp")
                pts = {}

                def emit_av(j, kg0):
                    pt, qs = pts.pop(j)
                    for h in range(2):
                        nc.tensor.matmul(
                            otp[:, h * TC + qs:(h + 1) * TC],
                            vaug[:, kg0 * 130 + h * 65:kg0 * 130 + h * 65 + 65],
                            pt[:, h * TC + qs:(h + 1) * TC],
                            start=(j == 0), stop=(j == n_kt - 1),
                            skip_group_check=True,
                        )

                for kt in range(n_kt):
                    kg = b * 16 + kt
                    diag = kt >= 4 * qc
                    v = kt - 4 * qc if diag else 0
                    qs = v * 128
                    sp = pspool.tile([128, 2 * TC], FP, tag="s", bufs=2,
                                     name="sp")
                    for h in range(2):
                        nc.tensor.matmul(
                            sp[:, h * TC + qs:(h + 1) * TC],
                            KT[h * 64:(h + 1) * 64, kg * 128:(kg + 1) * 128],
                            QT[h * 64:(h + 1) * 64, t0 + qs:t0 + TC],
                            start=True, stop=not diag,
                            skip_group_check=True,
                        )
                    if diag:
                        for h in range(2):
                            nc.tensor.matmul(
                                sp[:, h * TC + qs:h * TC + qs + 128],
                                ident[:],
                                mtri_sb[:],
                                start=False, stop=True,
                                skip_group_check=True,
                            )
                    pt = sbpool.tile([128, 2 * TC], BF, tag="pt", bufs=5,
                                     name="pt")
                    nc.scalar.activation(
                        pt[:].rearrange("p (j q) -> p j q", j=2)[:, :, qs:TC],
                        sp[:].rearrange("p (j q) -> p j q", j=2)[:, :, qs:TC],
                        ACT.Exp, scale=0.125,
                    )
                    pts[kt] = (pt, qs)
                    if fill:
                        fill.pop(0)()
                    if kt >= AV_DELAY:
                        emit_av(kt - AV_DELAY, b * 16 + kt - AV_DELAY)
                for j in range(max(n_kt - AV_DELAY, 0), n_kt):
                    emit_av(j, b * 16 + j)
                while fill:
                    fill.pop(0)()

                # rowsum extraction (ScalarE, fp32r-rounded); the rest of
                # the normalize is deferred into the next attention
                rc2 = sbpool.tile([1, 2 * TC], FPR, tag="rc", bufs=2,
                                  name="rc2")
                with nc.allow_low_precision(reason="softmax sums f32r"):
                    nc.scalar.copy(rc2[:], otp[64:65, :])
                state["pending"] = (otp, rc2, t0)

            # ---- global schedule ----
            # Q(b,0) Q(b,1) then attentions carry the next QKV phase plus
            # the previous attention's normalize+outproj as in-loop fill.
            q_units = {}                      # tcx -> qkv unit list
            q_units[0] = qkv_units(0, xtile_pre=xtile0)
            for u in q_units[0]:
                u()                           # chunk 0 QKV inline
            for u in qkv_units(1):
                u()                           # chunk 1 QKV inline

            # (attention tcx, qkv units of chunk to prefetch)
            sched = [(1, 2), (2, 3), (3, 4), (0, 5),
                     (5, 6), (6, 7), (7, None), (4, None)]
            for atc, qtc in sched:
                extra = qkv_units(qtc) if qtc is not None else []
                if state["pending"] is not None:
                    fill = drain_units(extra)
                else:
                    noop = lambda: None
                    fill = extra[:1] + [noop, noop, noop] + extra[1:]
                emit_attn(atc, fill)

            # final drain of the last attention
            otp_p, rc2_p, t0_p = state["pending"]
            ots_f = emit_norm(otp_p, rc2_p)
            for m in range(8):
                outproj_m(ots_f, t0_p, m)
    nc.compile()
    return nc


def make_in_maps(x, w_qkv, b_qkv, w_out):
    x = np.ascontiguousarray(np.asarray(x, np.float32).reshape(BT, C))
    xT = np.ascontiguousarray(x.T)                    # [C, BT]
    # [a(8), p(128), tcx(8), t(512)] -> [p, tcx, a, t]
    xhp = np.ascontiguousarray(
        xT.reshape(8, 128, NTC, TC).transpose(1, 2, 0, 3).reshape(128, -1)
    ).astype(ml_dtypes.bfloat16)
    w_qkv = np.asarray(w_qkv, np.float32)
    b_qkv = np.asarray(b_qkv, np.float32)
    w_out = np.asarray(w_out, np.float32)

    kk = np.arange(128)[:, None]
    qq = np.arange(128)[None, :]
    mtri = np.where(kk <= qq, 0.0, NEG).astype(ml_dtypes.bfloat16)

    in_maps = []
    for c in range(NCORES):
        sl = slice(c * 128, (c + 1) * 128)
        wcs = np.concatenate(
            [w_qkv[:, sl], w_qkv[:, 1024:][:, sl], w_qkv[:, 2048:][:, sl]],
            axis=1,
        )
        bq = np.stack(
            [b_qkv[sl], b_qkv[1024:][sl], b_qkv[2048:][sl]], axis=1
        )
        in_maps.append({
            "xh": xhp,
            "wc": np.ascontiguousarray(wcs).astype(ml_dtypes.bfloat16),
            "wout": np.ascontiguousarray(w_out[sl, :]).astype(
                ml_dtypes.bfloat16),
            "bqkv": np.ascontiguousarray(bq),
            "mtri": mtri,
            "ones": np.ones((128, 64), ml_dtypes.bfloat16),
            "onesr": np.ones((1, 64), np.float32),
        })
    return in_maps


_NC_CACHE = None


def kernel(x, w_qkv, b_qkv, w_out, b_out):
    global _NC_CACHE, LAST_RESULTS
    if _NC_CACHE is None:
        _NC_CACHE = build_nc()
    nc = _NC_CACHE

    in_maps = make_in_maps(x, w_qkv, b_qkv, w_out)

    res = run_bass_kernel_spmd(
        nc, in_maps, list(range(NCORES)),
        trace=bool(os.environ.get("BASS_TRACE")),
    )
    LAST_RESULTS = res

    acc = np.zeros((C, BT), np.float32)
    for out_map in res.results:
        # yh [p, tcx(8), m(8), t(512)] -> [m, p, tcx, t] -> [C, BT]
        yc = np.asarray(out_map["yh"]).reshape(128, NTC, 8, TC)
        acc += yc.transpose(2, 0, 1, 3).reshape(C, BT).astype(np.float32)
    y = acc.T + np.asarray(b_out, np.float32)[None, :]
    return y.reshape(B, T, C)


# revision 26
# speedup vs baseline: 1.1413x; 1.0929x over previous
"""Multi-head causal attention (B=2, T=2048, C=1024, H=16) on 8 Trainium2
NeuronCores, tensor-parallel over heads (2 heads per core).

v2 — engine-balanced redesign of the v1 flash kernel:
  - scores: the two heads' K^T@Q matmuls are issued adjacently with base
    partitions 0/64 so they land on different PE row groups and run
    CONCURRENTLY (row tiling), into the two halves of one [128,1024] PSUM
    pair tile.
  - causal mask: added on the PE as a tiny N=128 matmul (identity @ mtri)
    accumulated into the diagonal 128-block of the scores group — replaces
    the expensive [128,512] DVE adds.
  - exp: one ScalarE ACTIVATE per k-tile covering BOTH heads via a 3D AP
    over the [128,1024] pair (halves ScalarE instruction count).
  - diagonal k-tiles only compute the valid q-range (scores, exp, AV all
    use free dim 512-128*v).
  - AV keeps the ones-column trick (stationary [128,65] Vaug) for free
    softmax denominators; both heads' O accumulate across the whole chunk
    in two single-buffered PSUM banks.
  - normalize: rowsums -> reciprocal on [2,512] -> one fp32r broadcast
    matmul -> ScalarE copy -> two DVE muls into a combined ots[128,512]
    (h0 on partitions 0-63, h1 on 64-127).
  - out-projection: single full-contract matmul per m-tile (contract over
    both heads at once), DVE copy to bf16, DMA out.
  - x input host-packed as [p, chunk, ktile, t] so each chunk's load is one
    DMA with 8KB contiguous lines; y output in bf16 (halves write traffic).
"""

import os
import sys

for _p in ("/opt/trn_rl_repo", "/root/.axon_site/_ro/trn_rl_repo"):
    if os.path.isdir(_p) and _p not in sys.path:
        sys.path.insert(0, _p)

import ml_dtypes
import numpy as np

import concourse.bacc as bacc
import concourse.bass as bass
import concourse.mybir as mybir
import concourse.tile as tile
from concourse.bass_utils import run_bass_kernel_spmd
from concourse.masks import make_identity

B, T, C, H, D = 2, 2048, 1024, 16, 64
NCORES = 8
BT = B * T                      # 4096 flattened tokens
TC = 512                        # token chunk (matmul free dim)
NTC = BT // TC                  # 8 token chunks
FP = mybir.dt.float32
FPR = mybir.dt.float32r
BF = mybir.dt.bfloat16
ACT = mybir.ActivationFunctionType
NEG = -1.0e9
AV_DELAY = 3                    # k-tiles the AV matmul trails the scores

LAST_RESULTS = None             # stashed BassKernelResults for test harness


def build_nc():
    nc = bacc.Bacc(None, target_bir_lowering=False, debug=False)

    xh = nc.declare_dram_parameter("xh", [128, NTC * 4096], BF, isOutput=False)
    wc = nc.declare_dram_parameter("wc", [C, 384], BF, isOutput=False)
    wout = nc.declare_dram_parameter("wout", [128, C], BF, isOutput=False)
    bqkv = nc.declare_dram_parameter("bqkv", [128, 3], FP, isOutput=False)
    mtri = nc.declare_dram_parameter("mtri", [128, 128], BF, isOutput=False)
    ones = nc.declare_dram_parameter("ones", [128, 64], BF, isOutput=False)
    onesr = nc.declare_dram_parameter("onesr", [1, 64], FP, isOutput=False)
    yh = nc.declare_dram_parameter("yh", [128, NTC * 4096], BF, isOutput=True)

    with tile.TileContext(nc) as tc:
        with (
            tc.tile_pool(name="const", bufs=1) as cpool,
            tc.tile_pool(name="big", bufs=1) as bigpool,
            tc.tile_pool(name="sb", bufs=2) as sbpool,
            tc.tile_pool(name="ps", bufs=2, space="PSUM") as pspool,
        ):
            # ---- constants (wc group-0 + first x chunk lead the DMA ring
            # so the first matmuls can start ASAP) ----
            wc_sb = cpool.tile([128, 8 * 384], BF)      # [cin, k*384 + g*128 + col]
            wc_r = wc.rearrange("(a b) (g c) -> b a g c", a=8, g=3)
            wc_v = wc_sb[:].rearrange("b (a g c) -> b a g c", a=8, g=3)
            nc.sync.dma_start(out=wc_v[:, :, 0], in_=wc_r[:, :, 0])
            bq_sb = cpool.tile([128, 3], FP)
            nc.sync.dma_start(out=bq_sb[:], in_=bqkv[:, :])
            ident = cpool.tile([128, 128], BF)
            make_identity(nc, ident)

            # all x chunks are prefetched up front (SBUF has room); chunk 0
            # is split in halves so its first k-tiles land early
            xtiles = []
            for tcx in range(NTC):
                xt_ = sbpool.tile([128, 4096], BF, tag="xt", bufs=NTC,
                                  name=f"xtile{tcx}")
                xtiles.append(xt_)
            nc.sync.dma_start(out=xtiles[0][:, 0:2048], in_=xh[:, 0:2048])
            nc.sync.dma_start(out=wc_v[:, :, 1:3], in_=wc_r[:, :, 1:3])
            nc.sync.dma_start(out=xtiles[0][:, 2048:4096],
                              in_=xh[:, 2048:4096])
            mtri_sb = cpool.tile([128, 128], BF)
            nc.sync.dma_start(out=mtri_sb[:], in_=mtri[:, :])
            onesr_sb = cpool.tile([1, 64], FPR)
            nc.sync.dma_start(out=onesr_sb[:], in_=onesr.bitcast(FPR)[:, :])
            for tcx in (1, 2):
                nc.sync.dma_start(
                    out=xtiles[tcx][:],
                    in_=xh[:, tcx * 4096:(tcx + 1) * 4096],
                )
            wout_sb = cpool.tile([128, C], BF)          # rows: h0 d | h1 d
            nc.sync.dma_start(out=wout_sb[:], in_=wout[:, :])
            for tcx in range(3, NTC):
                nc.sync.dma_start(
                    out=xtiles[tcx][:],
                    in_=xh[:, tcx * 4096:(tcx + 1) * 4096],
                )

            # ---- persistent intermediates ----
            QT = bigpool.tile([128, BT], BF)
            KT = bigpool.tile([128, BT], BF)
            VT = bigpool.tile([128, BT], BF)
            # V in [token, dim] layout, 130 cols per 128-token block:
            # [V_h0 (64) | ones | V_h1 (64) | ones]
            vaug = bigpool.tile([128, 32 * 130], BF)
            ones_sb = cpool.tile([128, 64], BF)
            nc.sync.dma_start(out=ones_sb[:], in_=ones[:, :])
            nc.vector.tensor_copy(
                vaug[:].rearrange("p (j a c) -> p j a c", a=2, c=65)[
                    :, :, :, 64:65],
                ones_sb[:].rearrange("p (j a c) -> p j a c", a=2, c=1)[:, 0:32],
            )

            qkvT = (QT, KT, VT)
            state = {"pending": None}   # (otp, rc2, t0) awaiting norm+proj

            def qkv_group(tcx, xtile, g):
                qp = pspool.tile([128, TC], FP, tag="t", bufs=2, name="qp")
                for k in range(8):
                    nc.tensor.matmul(
                        qp[:],
                        wc_sb[:, k * 384 + g * 128:k * 384 + (g + 1) * 128],
                        xtile[:, k * TC:(k + 1) * TC],
                        start=(k == 0),
                        stop=(k == 7),
                    )
                nc.vector.tensor_scalar_add(
                    qkvT[g][:, tcx * TC:(tcx + 1) * TC], qp[:],
                    bq_sb[:, g:g + 1],
                )

            def v_transpose(tcx, j):
                jj = tcx * 4 + j
                tpf = pspool.tile([128, TC], FP, tag="t", bufs=2, name="tp")
                tp = tpf.bitcast(BF)[:, 0:128]
                nc.tensor.transpose(
                    tp, VT[:, jj * 128:(jj + 1) * 128], ident[:]
                )
                nc.vector.tensor_copy(
                    vaug[:].rearrange("p (j a c) -> p j a c", a=2, c=65)[
                        :, jj, :, 0:64],
                    tp.rearrange("p (a c) -> p a c", c=64),
                )

            def qkv_units(tcx):
                """fill-units (callables) computing chunk tcx's Q/K/V."""
                units = [lambda g=g: qkv_group(tcx, xtiles[tcx], g)
                         for g in range(3)]
                units.append(lambda: (v_transpose(tcx, 0),
                                      v_transpose(tcx, 1)))
                units.append(lambda: (v_transpose(tcx, 2),
                                      v_transpose(tcx, 3)))
                return units

            def emit_norm(otp_prev, rc2_prev):
                """normalize a finished chunk: fp32r broadcast matmuls of
                the rowsums, reciprocal, scale O -> ots (bf16)."""
                bcps = []
                for h in range(2):
                    bcp = pspool.tile([64, TC], FP, tag="t", bufs=2,
                                      name="bcp")
                    nc.tensor.matmul(
                        bcp[:], onesr_sb[:],
                        rc2_prev[0:1, h * TC:(h + 1) * TC],
                        start=True, stop=True, skip_group_check=True,
                    )
                    bcps.append(bcp)
                bcs = sbpool.tile([64, 2 * TC], FP, tag="bc", bufs=2,
                                  name="bcs")
                for h in range(2):
                    nc.vector.reciprocal_approx_fast(
                        out=bcs[:, h * TC:(h + 1) * TC], in_=bcps[h][:]
                    )
                ots = sbpool.tile([128, TC], BF, tag="ot", bufs=2, name="ots")
                nc.vector.tensor_mul(ots[0:64, :], otp_prev[0:64, 0:TC],
                                     bcs[0:64, 0:TC])
                nc.vector.tensor_mul(ots[64:128, :], otp_prev[0:64, TC:2 * TC],
                                     bcs[0:64, TC:2 * TC])
                return ots

            def outproj_m(ots_prev, t0_prev, m):
                yp = pspool.tile([128, TC], FP, tag="t", bufs=2, name="yp")
                nc.tensor.matmul(
                    yp[:], wout_sb[:, m * 128:(m + 1) * 128],
                    ots_prev[:], start=True, stop=True,
                )
                ysb = sbpool.tile([128, TC], BF, tag="ys", bufs=4, name="ysb")
                nc.vector.tensor_copy(ysb[:], yp[:])
                nc.sync.dma_start(
                    out=yh[:, (t0_prev // TC) * 4096
                           + m * TC:(t0_prev // TC) * 4096 + (m + 1) * TC],
                    in_=ysb[:],
                )

            def drain_units(extra):
                """fill units for the pending chunk's normalize+outproj,
                interleaved with `extra` (next chunk's qkv units)."""
                box = {}

                def norm():
                    otp_p, rc2_p, t0_p = state["pending"]
                    state["pending"] = None
                    box["ots"] = emit_norm(otp_p, rc2_p)
                    box["t0"] = t0_p

                projs = [lambda m=m: outproj_m(box["ots"], box["t0"], m)
                         for m in range(8)]
                ex = list(extra)
                noop = lambda: None
                # dense qkv work fills the attention-start valley (kt 0);
                # norm at kt 1; another qkv unit before the first outproj
                # so the norm's DVE chain completes before outproj's matmul
                units = [ex.pop(0) if ex else noop, norm,
                         ex.pop(0) if ex else noop,
                         projs[0], projs[1]]
                rest = projs[2:]
                while ex or rest:
                    if ex:
                        units.append(ex.pop(0))
                    if rest:
                        units.append(rest.pop(0))
                return units

            def emit_attn(tcx, fill):
                b, qc = divmod(tcx, 4)
                t0 = tcx * TC
                n_kt = 4 * (qc + 1)
                otp = pspool.tile([65, 2 * TC], FP, tag="o", bufs=1,
                                  name="otp")
                pts = {}

                def emit_av(j, kg0):
                    pt, qs = pts.pop(j)
                    for h in range(2):
                        nc.tensor.matmul(
                            otp[:, h * TC + qs:(h + 1) * TC],
                            vaug[:, kg0 * 130 + h * 65:kg0 * 130 + h * 65 + 65],
                            pt[:, h * TC + qs:(h + 1) * TC],
                            start=(j == 0), stop=(j == n_kt - 1),
                            skip_group_check=True,
                        )

                for kt in range(n_kt):
                    kg = b * 16 + kt
                    diag = kt >= 4 * qc
                    v = kt - 4 * qc if diag else 0
                    qs = v * 128
                    sp = pspool.tile([128, 2 * TC], FP, tag="s", bufs=2,
                                     name="sp")
                    for h in range(2):
                        nc.tensor.matmul(
                            sp[:, h * TC + qs:(h + 1) * TC],
                            KT[h * 64:(h + 1) * 64, kg * 128:(kg + 1) * 128],
                            QT[h * 64:(h + 1) * 64, t0 + qs:t0 + TC],
                            start=True, stop=not diag,
                            skip_group_check=True,
                        )
                    if diag:
                        for h in range(2):
                            nc.tensor.matmul(
                                sp[:, h * TC + qs:h * TC + qs + 128],
                                ident[:],
                                mtri_sb[:],
                                start=False, stop=True,
                                skip_group_check=True,
                            )
                    pt = sbpool.tile([128, 2 * TC], BF, tag="pt", bufs=5,
                                     name="pt")
                    nc.scalar.activation(
                        pt[:].rearrange("p (j q) -> p j q", j=2)[:, :, qs:TC],
                        sp[:].rearrange("p (j q) -> p j q", j=2)[:, :, qs:TC],
                        ACT.Exp, scale=0.125,
                    )
                    pts[kt] = (pt, qs)
                    if fill:
                        fill.pop(0)()
                    if kt >= AV_DELAY:
                        emit_av(kt - AV_DELAY, b * 16 + kt - AV_DELAY)
                for j in range(max(n_kt - AV_DELAY, 0), n_kt):
                    emit_av(j, b * 16 + j)
                while fill:
                    fill.pop(0)()

                # rowsum extraction (ScalarE, fp32r-rounded); the rest of
                # the normalize is deferred into the next attention
                rc2 = sbpool.tile([1, 2 * TC], FPR, tag="rc", bufs=2,
                                  name="rc2")
                with nc.allow_low_precision(reason="softmax sums f32r"):
                    nc.scalar.copy(rc2[:], otp[64:65, :])
                state["pending"] = (otp, rc2, t0)

            # ---- global schedule ----
            # Q(b,0) Q(b,1) then attentions carry the next QKV phase plus
            # the previous attention's normalize+outproj as in-loop fill.
            for u in qkv_units(0):
                u()                           # chunk 0 QKV inline
            for u in qkv_units(1):
                u()                           # chunk 1 QKV inline

            # (attention tcx, qkv units of chunk to prefetch)
            sched = [(1, 2), (2, 3), (3, 4), (0, 5),
                     (5, 6), (6, 7), (7, None), (4, None)]
            for atc, qtc in sched:
                extra = qkv_units(qtc) if qtc is not None else []
                if state["pending"] is not None:
                    fill = drain_units(extra)
                else:
                    fill = list(extra)
                emit_attn(atc, fill)

            # final drain of the last attention
            otp_p, rc2_p, t0_p = state["pending"]
            ots_f = emit_norm(otp_p, rc2_p)
            for m in range(8):
                outproj_m(ots_f, t0_p, m)
    nc.compile()
    return nc


def make_in_maps(x, w_qkv, b_qkv, w_out):
    x = np.ascontiguousarray(np.asarray(x, np.float32).reshape(BT, C))
    xT = np.ascontiguousarray(x.T)                    # [C, BT]
    # [a(8), p(128), tcx(8), t(512)] -> [p, tcx, a, t]
    xhp = np.ascontiguousarray(
        xT.reshape(8, 128, NTC, TC).transpose(1, 2, 0, 3).reshape(128, -1)
    ).astype(ml_dtypes.bfloat16)
    w_qkv = np.asarray(w_qkv, np.float32)
    b_qkv = np.asarray(b_qkv, np.float32)
    w_out = np.asarray(w_out, np.float32)

    kk = np.arange(128)[:, None]
    qq = np.arange(128)[None, :]
    mtri = np.where(kk <= qq, 0.0, NEG).astype(ml_dtypes.bfloat16)

    in_maps = []
    for c in range(NCORES):
        sl = slice(c * 128, (c + 1) * 128)
        wcs = np.concatenate(
            [w_qkv[:, sl], w_qkv[:, 1024:][:, sl], w_qkv[:, 2048:][:, sl]],
            axis=1,
        )
        bq = np.stack(
            [b_qkv[sl], b_qkv[1024:][sl], b_qkv[2048:][sl]], axis=1
        )
        in_maps.append({
            "xh": xhp,
            "wc": np.ascontiguousarray(wcs).astype(ml_dtypes.bfloat16),
            "wout": np.ascontiguousarray(w_out[sl, :]).astype(
                ml_dtypes.bfloat16),
            "bqkv": np.ascontiguousarray(bq),
            "mtri": mtri,
            "ones": np.ones((128, 64), ml_dtypes.bfloat16),
            "onesr": np.ones((1, 64), np.float32),
        })
    return in_maps


_NC_CACHE = None


def kernel(x, w_qkv, b_qkv, w_out, b_out):
    global _NC_CACHE, LAST_RESULTS
    if _NC_CACHE is None:
        _NC_CACHE = build_nc()
    nc = _NC_CACHE

    in_maps = make_in_maps(x, w_qkv, b_qkv, w_out)

    res = run_bass_kernel_spmd(
        nc, in_maps, list(range(NCORES)),
        trace=bool(os.environ.get("BASS_TRACE")),
    )
    LAST_RESULTS = res

    acc = np.zeros((C, BT), np.float32)
    for out_map in res.results:
        # yh [p, tcx(8), m(8), t(512)] -> [m, p, tcx, t] -> [C, BT]
        yc = np.asarray(out_map["yh"]).reshape(128, NTC, 8, TC)
        acc += yc.transpose(2, 0, 1, 3).reshape(C, BT).astype(np.float32)
    y = acc.T + np.asarray(b_out, np.float32)[None, :]
    return y.reshape(B, T, C)
